# revision 14
# baseline (speedup 1.0000x reference)
"""Grouped-Query Attention (B=2, S=2048, D=2048, 16 Q heads / 4 KV heads,
hd=128, RoPE, causal) on 8 trn2 NeuronCores.

Sharding: mesh = 2 (batch) x 4 (KV-head groups).  Core c = b*4 + g gets
batch b and KV head g together with its 4 query heads (tensor parallel on
the head dim: q/k/v projection output dim and o-proj input dim).  Each core
produces a partial y[b] (o-proj over its 512 input dims, emitted bf16);
host sums the 4 partials per batch in fp32.

On-chip layout: all activations transposed ([feature, seq]) so every matmul
contracts along the partition dim.  Matmuls run as float32r / bf16
(1 cycle/row at >=256 free columns).  Softmax is unnormalized in scoresT
[sk, sq] orientation: exp(scale*s) on ACT, causal masking as a 0/1 multiply
on the exp output in SBUF (DVE/GpSimd), denominator via ones-column matmul
accumulated in PSUM, 1/den via a single custom-DVE reciprocal_approx_fast,
normalization via K=1 broadcast matmul + DVE multiply.

v2 scheduling notes:
  * startup DMAs are ordered critical-first per queue (first weight chunks
    and the first x tile ahead of everything; tables stream during sb0);
  * RoPE rotate-half is done with quadrant-shifted DVE reads against a
    pre-shifted sin table (no PE shift matmul, no PSUM rot tiles);
  * phase 2 runs one (sq block, head) section at a time: per kb the PE does
    scores + av + den (~640ns) while ACT does one exp (~720ns); deferred
    o-proj chains drain into the slack so the PE never waits on ACT;
  * 1/den runs on DVE (reciprocal_approx_fast), keeping ACT exp-only;
  * PSUM: scores ring 3 banks, av 2, den 1, bc/o-proj ring 2 = 8;
  * y is written bf16 on two DMA queues to shrink the output tail.
"""

import os

import numpy as np

S = 2048
D = 2048
HD = 128
NQH = 16
NKVH = 4
GROUPS = NQH // NKVH  # 4 q heads per kv head
O = GROUPS * HD  # 512 per-core q/o slice
NB = 2
NCORES = 8
SCALE = 1.0 / float(np.sqrt(np.float32(HD)))

SBLK = 512  # seq block for projections / sq block in attention
NKB = S // HD  # 16 128-blocks along seq
NSB = S // SBLK  # 4 512-blocks along seq
NDB = D // HD  # 16 d blocks

LAST_EXEC_NS = None
LAST_TRACE = None

_CACHE = {}


def _rope_tables():
    k = np.arange(0, HD, 2)[: HD // 2].astype(np.float32)
    inv_freq = (1.0 / 10000.0 ** (k / HD)).astype(np.float32)
    positions = np.arange(S, dtype=np.float32)
    ang = positions[:, None] * inv_freq[None, :]  # [S, 64]
    ang = np.concatenate([ang, ang], axis=-1)  # [S, 128]
    cosT = np.cos(ang).astype(np.float32).T  # [128, S]
    sinT = np.sin(ang).astype(np.float32).T
    return np.ascontiguousarray(cosT), np.ascontiguousarray(sinT)


def _shift_table():
    # rot = P @ q  with rot[i] = -q[i+64] (i<64), q[i-64] (i>=64); ship P.T
    P = np.zeros((HD, HD), dtype=np.float32)
    h = HD // 2
    P[np.arange(h), np.arange(h) + h] = -1.0
    P[np.arange(h) + h, np.arange(h)] = 1.0
    return np.ascontiguousarray(P.T)


def _mask_table():
    # binary keep-mask: maskT[i, j*512 + s] = 1 if (j*128 + i) <= s else 0
    m = np.empty((HD, 4 * SBLK), dtype=np.float32)
    i = np.arange(HD)[:, None]
    s = np.arange(SBLK)[None, :]
    for j in range(4):
        m[:, j * SBLK : (j + 1) * SBLK] = np.where(j * HD + i <= s, 1.0, 0.0)
    return m


def _build_program(split_waits=True):
    import concourse.bass as bass
    import concourse.mybir as mybir
    from concourse.tile import TileContext

    f32 = mybir.dt.float32
    f32r = mybir.dt.float32r
    bf16 = mybir.dt.bfloat16
    EXP = mybir.ActivationFunctionType.Exp
    LN = mybir.ActivationFunctionType.Ln

    nc = bass.Bass()

    xP = nc.declare_dram_parameter("xP", [NSB * 4 * 128, 4 * SBLK], bf16, isOutput=False)
    wqP = nc.declare_dram_parameter("wqP", [NDB * 128, O], bf16, isOutput=False)
    wkP = nc.declare_dram_parameter("wkP", [128, NDB * HD], bf16, isOutput=False)
    wvP = nc.declare_dram_parameter("wvP", [128, NDB * HD], bf16, isOutput=False)
    woP = nc.declare_dram_parameter("woP", [128, GROUPS * D], bf16, isOutput=False)
    cosB = nc.declare_dram_parameter("cosB", [HD, S], bf16, isOutput=False)
    sinB = nc.declare_dram_parameter("sinB", [HD, S], bf16, isOutput=False)
    shiftPT = nc.declare_dram_parameter("shiftPT", [HD, HD], f32r, isOutput=False)
    maskT = nc.declare_dram_parameter("maskT", [HD, 4 * SBLK], f32, isOutput=False)
    ident = nc.declare_dram_parameter("ident", [HD, HD], f32r, isOutput=False)
    onescol = nc.declare_dram_parameter("onescol", [HD, 1], f32r, isOutput=False)
    onesrow = nc.declare_dram_parameter("onesrow", [1, HD], f32r, isOutput=False)
    y = nc.declare_dram_parameter("y", [S, D], bf16, isOutput=True)

    with TileContext(nc) as tc:
        with tc.tile_pool(name="persist", bufs=1) as pp:
            wq_sb = pp.tile([128, NDB * O], bf16, name="wq_sb")  # [d_blk][128d, 512o]
            wk_sb = pp.tile([128, NDB * HD], bf16, name="wk_sb")
            wv_sb = pp.tile([128, NDB * HD], bf16, name="wv_sb")
            wo_sb = pp.tile([128, GROUPS * D], bf16, name="wo_sb")  # [o_blk][128o, 2048]
            cos_sb = pp.tile([128, S], f32, name="cos_sb")
            sin_sb = pp.tile([128, S], f32, name="sin_sb")
            shift_sb = pp.tile([128, HD], f32r, name="shift_sb")
            mask_sb = pp.tile([128, 4 * SBLK], f32, name="mask_sb")
            id_sb = pp.tile([128, HD], f32r, name="id_sb")
            ones_sb = pp.tile([128, 1], f32r, name="ones_sb")
            oner_sb = pp.tile([1, HD], f32r, name="oner_sb")
            # per-sb tiles so phase-2 readers only depend on the blocks
            # they actually touch (Tile tracks deps at tile granularity)
            q_sbs = [
                pp.tile([128, GROUPS * SBLK], f32r, name=f"q_sb{sb}")
                for sb in range(NSB)
            ]  # [h][128hd, 512s]
            k_sbs = [pp.tile([128, SBLK], f32r, name=f"k_sb{sb}") for sb in range(NSB)]
            v_sbs = [
                pp.tile([128, 4 * HD], f32r, name=f"v_sb{sb}") for sb in range(NSB)
            ]  # [sub][128s, 128hd]

            # Critical prefix on the scalar queue: exactly what the first
            # few projection blocks need; everything else streams during sb0.
            nc.scalar.dma_start(out=wq_sb[:, 0:O], in_=wqP[0:128, :])
            nc.scalar.dma_start(out=wk_sb[:, 0 : 4 * HD], in_=wkP[:, 0 : 4 * HD])
            nc.scalar.dma_start(out=wv_sb[:, 0 : 4 * HD], in_=wvP[:, 0 : 4 * HD])

            # ---------------- Phase 1: projections + RoPE + v transpose
            with (
                tc.tile_pool(name="p1acc", bufs=6, space="PSUM") as accp,
                tc.tile_pool(name="p1rot", bufs=2, space="PSUM") as rotp,
                tc.tile_pool(name="xts", bufs=6) as xpool,
                tc.tile_pool(name="raws", bufs=4) as rawpool,
                tc.tile_pool(name="tmps", bufs=2) as tmppool,
            ):
                p1_pending = []
                p2_warm = []
                rope_pool = [rotp]
                tmp_pool = [tmppool]

                def p1_flush(n=99):
                    nonlocal p1_pending
                    k = min(n, len(p1_pending))
                    for f in p1_pending[:k]:
                        f()
                    p1_pending = p1_pending[k:]

                for sb in range(NSB):
                    sl = slice(sb * SBLK, (sb + 1) * SBLK)
                    ps = [
                        accp.tile([128, SBLK], f32, name=f"acc{i}_{sb}", tag="acc")
                        for i in range(6)
                    ]  # q0..q3, k, v
                    xt4 = None
                    for db in range(NDB):
                        # staged weight/table DMAs, queue-ordered critical-first
                        if sb == 0:
                            if db == 1:
                                nc.scalar.dma_start(
                                    out=wq_sb[:, O : 4 * O],
                                    in_=wqP[128 : 4 * 128, :].rearrange(
                                        "(g p) c -> p g c", g=3
                                    ),
                                )
                            elif db == 2:
                                nc.scalar.dma_start(
                                    out=wq_sb[:, 4 * O : 8 * O],
                                    in_=wqP[4 * 128 : 8 * 128, :].rearrange(
                                        "(g p) c -> p g c", g=4
                                    ),
                                )
                            elif db == 3:
                                nc.scalar.dma_start(
                                    out=wk_sb[:, 4 * HD :],
                                    in_=wkP[:, 4 * HD :],
                                )
                                nc.scalar.dma_start(
                                    out=wv_sb[:, 4 * HD :],
                                    in_=wvP[:, 4 * HD :],
                                )
                            elif db == 6:
                                nc.scalar.dma_start(
                                    out=wq_sb[:, 8 * O : 12 * O],
                                    in_=wqP[8 * 128 : 12 * 128, :].rearrange(
                                        "(g p) c -> p g c", g=4
                                    ),
                                )
                            elif db == 8:
                                nc.scalar.dma_start(
                                    out=wq_sb[:, 12 * O : 16 * O],
                                    in_=wqP[12 * 128 : 16 * 128, :].rearrange(
                                        "(g p) c -> p g c", g=4
                                    ),
                                )
                            elif db == 10:
                                cosb = xpool.tile([128, S], bf16, name="cosb", tag="xt")
                                nc.scalar.dma_start(out=cosb[:], in_=cosB[:])
                                nc.vector.tensor_copy(cos_sb[:], cosb[:])
                            elif db == 11:
                                sinb = xpool.tile([128, S], bf16, name="sinb", tag="xt")
                                nc.scalar.dma_start(out=sinb[:], in_=sinB[:])
                                nc.vector.tensor_copy(sin_sb[:], sinb[:])
                            elif db == 13:
                                nc.scalar.dma_start(out=shift_sb[:], in_=shiftPT[:])
                                nc.scalar.dma_start(out=id_sb[:], in_=ident[:])
                                nc.scalar.dma_start(out=ones_sb[:], in_=onescol[:])
                                nc.scalar.dma_start(out=oner_sb[:], in_=onesrow[:])
                        elif sb == 1:
                            if db in (2, 6, 10, 14):
                                c = (db - 2) // 4
                                nc.scalar.dma_start(
                                    out=wo_sb[:, c * 2048 : (c + 1) * 2048],
                                    in_=woP[:, c * 2048 : (c + 1) * 2048],
                                )
                            elif db == 12:
                                nc.scalar.dma_start(out=mask_sb[:], in_=maskT[:])
                        if db % 4 == 0:
                            g = db // 4
                            xt4 = xpool.tile(
                                [128, 4 * SBLK], bf16, name=f"xt{sb}_{g}", tag="xt"
                            )
                            xq = nc.sync if (sb == 0 or g % 2 == 0) else nc.scalar
                            blk = (sb * 4 + g) * 128
                            if sb == 0 and g == 0:
                                xq.dma_start(
                                    out=xt4[:, 0:SBLK], in_=xP[blk : blk + 128, 0:SBLK]
                                )
                                xq.dma_start(
                                    out=xt4[:, SBLK:], in_=xP[blk : blk + 128, SBLK:]
                                )
                            else:
                                xq.dma_start(out=xt4[:], in_=xP[blk : blk + 128, :])
                        xt = xt4[:, (db % 4) * SBLK : (db % 4 + 1) * SBLK]
                        st = db == 0
                        sp = db == NDB - 1
                        if db > 1 and db % 2 == 0:
                            p1_flush(1)
                        for ob in range(GROUPS):
                            nc.tensor.matmul(
                                ps[ob][:],
                                wq_sb[:, db * O + ob * 128 : db * O + (ob + 1) * 128],
                                xt,
                                start=st,
                                stop=sp,
                            )
                        nc.tensor.matmul(
                            ps[4][:],
                            wk_sb[:, db * HD : (db + 1) * HD],
                            xt,
                            start=st,
                            stop=sp,
                        )
                        nc.tensor.matmul(
                            ps[5][:],
                            wv_sb[:, db * HD : (db + 1) * HD],
                            xt,
                            start=st,
                            stop=sp,
                        )

                    # PSUM -> SBUF copies spread across ACT/DVE (GpSimd
                    # cannot read PSUM) so the acc banks free up fast.
                    if sb == NSB - 1:
                        vst = pp.tile([128, SBLK], f32r, name=f"vst{sb}")
                    else:
                        vst = rawpool.tile([128, SBLK], f32r, name=f"vst{sb}", tag="vst", bufs=2)
                    nc.vector.tensor_copy(vst[:], ps[5][:])
                    raws = []
                    for i in range(5):
                        if sb == NSB - 1:
                            raw = pp.tile([128, SBLK], f32r, name=f"raw{sb}_{i}")
                        else:
                            raw = rawpool.tile([128, SBLK], f32r, name=f"raw{sb}_{i}", tag="raw", bufs=5)
                        if i % 2 == 0:
                            nc.scalar.copy(raw[:], ps[i][:])
                        else:
                            nc.vector.tensor_copy(raw[:], ps[i][:])
                        raws.append(raw)

                    # RoPE rotate (PE shift matmul) + v transposes go on the
                    # phase-1 work queue, drained one per db of the next sb so
                    # the PE never idles at block boundaries.  t2 = rot*sin on
                    # DVE (reads PSUM), t1 = raw*cos and adds on GpSimd.
                    def make_rope(sb, i, raw, sl=sl):
                        def run():
                            rot = rope_pool[0].tile(
                                [128, SBLK], f32, name=f"rot{sb}_{i}", tag="ring"
                            )
                            nc.tensor.matmul(
                                rot[:], shift_sb[:], raw[:], start=True, stop=True
                            )
                            dst = (
                                q_sbs[sb][:, i * SBLK : (i + 1) * SBLK]
                                if i < 4
                                else k_sbs[sb][:]
                            )
                            t1 = tmp_pool[0].tile(
                                [128, SBLK], f32, name=f"t1_{sb}_{i}", tag="t1", bufs=3
                            )
                            nc.gpsimd.tensor_mul(t1[:], raw[:], cos_sb[:, sl])
                            t2 = tmp_pool[0].tile(
                                [128, SBLK], f32, name=f"t2_{sb}_{i}", tag="t2", bufs=3
                            )
                            nc.vector.tensor_mul(t2[:], rot[:], sin_sb[:, sl])
                            (nc.gpsimd if i % 2 == 0 else nc.vector).tensor_add(
                                dst, t1[:], t2[:]
                            )
                        return run

                    def make_vt(sb, sub, vst=vst):
                        def run():
                            vt = rope_pool[0].tile(
                                [128, SBLK], f32r, name=f"vt{sb}_{sub}", tag="ring"
                            )
                            nc.tensor.transpose(
                                vt[:, :HD], vst[:, sub * HD : (sub + 1) * HD], id_sb[:]
                            )
                            nc.scalar.copy(
                                v_sbs[sb][:, sub * HD : (sub + 1) * HD], vt[:, :HD]
                            )
                        return run

                    if sb == NSB - 1:
                        # the last block's rope/vt drain INSIDE phase 2 (the
                        # data is only read by the late sq=3 sections), so
                        # the PE never idles across the phase boundary
                        for i in range(5):
                            p2_warm.append(make_rope(sb, i, raws[i]))
                        for sub in range(SBLK // HD):
                            p2_warm.append(make_vt(sb, sub))
                    else:
                        for i in range(5):
                            p1_pending.append(make_rope(sb, i, raws[i]))
                        for sub in range(SBLK // HD):
                            p1_pending.append(make_vt(sb, sub))
                p1_flush()

            # ---------------- Phase 2: attention + o-proj
            # kb-PAIRED softmax: scores for two kb blocks land in one
            # 2-bank PSUM tile and a single ACT exp covers both (amortizes
            # the ~290ns ACT per-op overhead).  PSUM: pair ring 2x2 banks,
            # av 1, den 1, bc/o-proj/rope ring 2 = 8.
            with (
                tc.tile_pool(name="p2pair", bufs=2, space="PSUM") as pairp,
                tc.tile_pool(name="p2av", bufs=1, space="PSUM") as avp,
                tc.tile_pool(name="p2den", bufs=1, space="PSUM") as denp,
                tc.tile_pool(name="p2ring", bufs=2, space="PSUM") as ringp,
                tc.tile_pool(name="exps", bufs=3) as epool,
                tc.tile_pool(name="denrs", bufs=2) as drpool,
                tc.tile_pool(name="ysb", bufs=3) as ypool_sb,
                tc.tile_pool(name="ystash", bufs=16) as ystash,
            ):
                rope_pool[0] = ringp
                tmp_pool[0] = drpool
                ao_bufs = [
                    pp.tile([128, GROUPS * SBLK], bf16, name=f"aobuf{i}")
                    for i in range(2)
                ]
                # deferred work drained into the following sections'
                # instruction streams: epilogues (hi) promptly, o-proj
                # chains (lo) as PE filler while ACT streams exps.
                pending_hi = []
                pending_lo = []  # (required epi_drained, closure)
                epi = [0, 0]  # emitted, drained

                def flush_hi(n=99):
                    nonlocal pending_hi
                    k = min(n, len(pending_hi))
                    for f in pending_hi[:k]:
                        f()
                    pending_hi = pending_hi[k:]
                    epi[1] += k

                def flush_lo(n):
                    nonlocal pending_lo
                    k = 0
                    while k < n and k < len(pending_lo) and pending_lo[k][0] <= epi[1]:
                        pending_lo[k][1]()
                        k += 1
                    pending_lo = pending_lo[k:]

                def drain(n):
                    # warm items (last sb's rope) first, then ready chains
                    k = min(n, len(p2_warm))
                    for f in p2_warm[:k]:
                        f()
                    del p2_warm[:k]
                    flush_lo(n - k)

                def make_chain(sq, sub, dc, ao, obs=(0, 1, 2, 3), stash=None):
                    def run():
                        yt = ringp.tile(
                            [128, SBLK], f32, name=f"y{sq}_{sub}_{dc}", tag="ring"
                        )
                        for ob in obs:
                            nc.tensor.matmul(
                                yt[:],
                                ao[:, ob * SBLK + sub * HD : ob * SBLK + (sub + 1) * HD],
                                wo_sb[:, ob * D + dc * SBLK : ob * D + (dc + 1) * SBLK],
                                start=(ob == obs[0]),
                                stop=(ob == obs[-1]),
                            )
                        if stash is not None and stash[0] is None:
                            # first half of a split chain: park it in SBUF
                            ys = ystash.tile(
                                [128, SBLK], bf16, name=f"yh{sub}_{dc}", tag="yh"
                            )
                            nc.vector.tensor_copy(ys[:], yt[:])
                            stash[0] = ys
                            return
                        ysb = ypool_sb.tile(
                            [128, SBLK], bf16, name=f"ysb{sq}_{sub}_{dc}", tag="ysb"
                        )
                        if stash is not None:
                            nc.vector.tensor_add(ysb[:], yt[:], stash[0][:])
                        else:
                            nc.vector.tensor_copy(ysb[:], yt[:])
                        yq = nc.sync if dc % 2 == 0 else nc.scalar
                        yq.dma_start(
                            out=y[
                                sq * SBLK + sub * HD : sq * SBLK + (sub + 1) * HD,
                                dc * SBLK : (dc + 1) * SBLK,
                            ],
                            in_=ysb[:],
                        )
                    return run

                # sq0 (2 pair iterations per head) is too thin to hide its
                # own seams; run it last so sq3's o-proj chains pad it, and
                # split its o-proj by head pair so only half the chains land
                # after the final section.
                for sq in (1, 2, 3, 0):
                    nsk = 4 * sq + 4
                    npr = nsk // 2
                    lag = 2 if npr >= 4 else 1
                    ao = ao_bufs[sq % 2]
                    stashes = {}
                    for h in range(GROUPS):
                        dent = denp.tile(
                            [1, SBLK], f32, name=f"den{sq}_{h}", tag="den"
                        )
                        avt = avp.tile(
                            [128, SBLK], f32, name=f"av{sq}_{h}", tag="av"
                        )
                        es = {}

                        def emit_avden(p, avt=avt, dent=dent, es=es, nsk=nsk):
                            e = es.pop(p)
                            for half in range(2):
                                kb = 2 * p + half
                                st = kb == 0
                                sp = kb == nsk - 1
                                esl = e[:, half * SBLK : (half + 1) * SBLK]
                                nc.tensor.matmul(
                                    avt[:],
                                    v_sbs[kb // 4][:, (kb % 4) * HD : (kb % 4 + 1) * HD],
                                    esl,
                                    start=st,
                                    stop=sp,
                                )
                                nc.tensor.matmul(
                                    dent[:],
                                    ones_sb[:],
                                    esl,
                                    start=st,
                                    stop=sp,
                                )

                        for p in range(npr):
                            pair = pairp.tile(
                                [128, 2 * SBLK], f32, name=f"pr{sq}_{h}_{p}", tag="pair"
                            )
                            for half in range(2):
                                kb = 2 * p + half
                                nc.tensor.matmul(
                                    pair[:, half * SBLK : (half + 1) * SBLK],
                                    k_sbs[kb // 4][:, (kb % 4) * HD : (kb % 4 + 1) * HD],
                                    q_sbs[sq][:, h * SBLK : (h + 1) * SBLK],
                                    start=True,
                                    stop=True,
                                )
                            e = epool.tile(
                                [128, 2 * SBLK], f32r, name=f"e{sq}_{h}_{p}", tag="e"
                            )
                            nc.scalar.activation(e[:], pair[:], EXP, scale=SCALE)
                            if 2 * p >= 4 * sq:
                                # causal mask on both halves at once; the
                                # final pair is latency-critical - keep it
                                # off the slow GpSimd
                                j = 2 * p - 4 * sq
                                meng = nc.vector if j else nc.gpsimd
                                meng.tensor_mul(
                                    e[:], e[:], mask_sb[:, j * SBLK : (j + 2) * SBLK]
                                )
                            es[p] = e
                            if p == 1:
                                flush_hi(1)
                            elif sq == 0:
                                drain(2)
                            elif len(pending_lo) > 8 or p % 2 == 0:
                                drain(1)
                            if p > lag - 1:
                                emit_avden(p - lag)
                        for p in range(npr - lag, npr):
                            emit_avden(p)
                            if p < npr - 1:
                                drain(1)

                        def epilogue(h=h, dent=dent, avt=avt, ao=ao):
                            # 1/den on ACT: denr = exp(-ln(den)); ln+exp live
                            # in the same ACT table set, so no table reloads.
                            denr = drpool.tile(
                                [1, SBLK], f32r, name=f"denr{h}", tag="denr", bufs=2
                            )
                            nc.scalar.activation(denr[:], dent[:], LN)
                            nc.scalar.activation(denr[:], denr[:], EXP, scale=-1.0)
                            bc = ringp.tile([128, SBLK], f32, name=f"bc{h}", tag="ring")
                            nc.tensor.matmul(
                                bc[:],
                                oner_sb[:],
                                denr[:],
                                start=True,
                                stop=True,
                            )
                            # TensorTensor may read only one PSUM operand:
                            # stage the broadcast through SBUF.
                            bcs = drpool.tile(
                                [128, SBLK], f32, name=f"bcs{h}", tag="bcs", bufs=2
                            )
                            nc.vector.tensor_copy(bcs[:], bc[:])
                            nc.vector.tensor_mul(
                                ao[:, h * SBLK : (h + 1) * SBLK], avt[:], bcs[:]
                            )

                        pending_hi.append(epilogue)
                        epi[0] += 1

                        if sq == 0 and h == 1:
                            # first half of the final block's o-proj: heads
                            # 0-1 are final once their epilogues drain
                            flush_hi()
                            for sub in range(SBLK // HD):
                                for dc in range(D // SBLK):
                                    stash = [None]
                                    stashes[(sub, dc)] = stash
                                    pending_lo.append(
                                        (epi[0], make_chain(sq, sub, dc, ao, (0, 1), stash))
                                    )

                    # o-proj chains for this sq block drain inside the next
                    # block's attention stream.
                    if sq == 0:
                        flush_hi()
                        for sub in range(SBLK // HD):
                            for dc in range(D // SBLK):
                                pending_lo.append(
                                    (epi[0], make_chain(
                                        sq, sub, dc, ao, (2, 3), stashes[(sub, dc)]
                                    ))
                                )
                    else:
                        for sub in range(SBLK // HD):
                            for dc in range(D // SBLK):
                                pending_lo.append(
                                    (epi[0], make_chain(sq, sub, dc, ao))
                                )
                flush_hi()
                drain(999)
    if split_waits:
        _split_matmul_waits(nc, mybir)
    return nc


def _split_matmul_waits(nc, mybir):
    """TRN2 instructions can carry only one HW sync-wait command; Tile
    sometimes attaches several.  Move the extras onto nofuse nops on the
    same engine inserted just before the instruction."""
    for f in nc.m.functions:
        for bb in f.blocks:
            insts = bb.instructions
            fixes = []
            for idx, inst in enumerate(insts):
                si = inst.sync_info
                if si is None or len(si.on_wait) <= 1:
                    continue
                fixes.append((idx, inst, list(si.on_wait), list(si.on_update)))
            for idx, inst, waits, updates in reversed(fixes):
                inst.sync_info = mybir.SyncInfo(on_wait=[waits[-1]], on_update=updates)
                for w in reversed(waits[:-1]):
                    nop = mybir.InstNoOp(
                        name=nc.get_next_instruction_name(), ins=[], outs=[]
                    )
                    nop.engine = inst.engine
                    nop.bass_nofuse = True
                    nop.sync_info = mybir.SyncInfo(on_wait=[w], on_update=[])
                    insts.insert(idx, nop)


def _per_core_inputs(x, Wq, Wk, Wv, Wo):
    import ml_dtypes

    bf16 = ml_dtypes.bfloat16
    cosT, sinT = _rope_tables()
    shiftPT = _shift_table()
    maskT = _mask_table()
    ident = np.eye(HD, dtype=np.float32)
    onescol = np.ones((HD, 1), dtype=np.float32)
    onesrow = np.ones((1, HD), dtype=np.float32)
    in_maps = []
    for b in range(NB):
        xTb = x[b].T.astype(bf16)  # [D, S]
        # pack into contiguous [sb][db][128, 512] blocks for sequential DMA
        xPb = np.ascontiguousarray(
            xTb.reshape(4, 4, 128, NSB, SBLK)      # [g, db_in_g, 128, sb, 512]
            .transpose(3, 0, 2, 1, 4)              # [sb, g, 128, db_in_g, 512]
            .reshape(NSB * 4 * 128, 4 * SBLK)
        )
        for g in range(NKVH):
            wqT = Wq[g * O : (g + 1) * O, :].T  # [D, O]
            wkT = Wk[g * HD : (g + 1) * HD, :].T
            wvT = Wv[g * HD : (g + 1) * HD, :].T
            woT = Wo[:, g * O : (g + 1) * O].T  # [O, D]
            in_maps.append(
                {
                    "xP": xPb,
                    "wqP": np.ascontiguousarray(
                        wqT.reshape(NDB * 128, O).astype(bf16)
                    ),
                    "wkP": np.ascontiguousarray(
                        wkT.reshape(NDB, 128, HD).transpose(1, 0, 2).reshape(128, NDB * HD).astype(bf16)
                    ),
                    "wvP": np.ascontiguousarray(
                        wvT.reshape(NDB, 128, HD).transpose(1, 0, 2).reshape(128, NDB * HD).astype(bf16)
                    ),
                    "woP": np.ascontiguousarray(
                        woT.reshape(GROUPS, 128, D).transpose(1, 0, 2).reshape(128, GROUPS * D).astype(bf16)
                    ),
                    "cosB": cosT.astype(bf16),
                    "sinB": sinT.astype(bf16),
                    "shiftPT": shiftPT,
                    "maskT": maskT,
                    "ident": ident,
                    "onescol": onescol,
                    "onesrow": onesrow,
                }
            )
    return in_maps


def kernel(x, Wq, Wk, Wv, Wo):
    global LAST_EXEC_NS, LAST_TRACE
    from concourse.bass_utils import run_bass_kernel_spmd

    if "nc" not in _CACHE:
        _CACHE["nc"] = _build_program()
    nc = _CACHE["nc"]

    x = np.asarray(x)
    in_maps = _per_core_inputs(
        x, np.asarray(Wq), np.asarray(Wk), np.asarray(Wv), np.asarray(Wo)
    )
    trace = bool(os.environ.get("KERNEL_PROFILE"))
    res = run_bass_kernel_spmd(
        nc, in_maps, core_ids=list(range(NCORES)), trace=trace
    )
    globals()["LAST_RESULT"] = res
    LAST_EXEC_NS = res.exec_time_ns
    LAST_TRACE = getattr(res, "profile_json", None)
    out = np.empty((NB, S, D), dtype=np.float32)
    for b in range(NB):
        acc = res.results[b * NKVH]["y"].astype(np.float32)
        for g in range(1, NKVH):
            acc = acc + res.results[b * NKVH + g]["y"].astype(np.float32)
        out[b] = acc
    return out


# revision 15
# speedup vs baseline: 1.0284x; 1.0284x over previous
"""Grouped-Query Attention (B=2, S=2048, D=2048, 16 Q heads / 4 KV heads,
hd=128, RoPE, causal) on 8 trn2 NeuronCores.

Sharding: mesh = 2 (batch) x 4 (KV-head groups).  Core c = b*4 + g gets
batch b and KV head g together with its 4 query heads (tensor parallel on
the head dim: q/k/v projection output dim and o-proj input dim).  Each core
produces a partial y[b] (o-proj over its 512 input dims, emitted bf16);
host sums the 4 partials per batch in fp32.

On-chip layout: all activations transposed ([feature, seq]) so every matmul
contracts along the partition dim.  Matmuls run as float32r / bf16
(1 cycle/row at >=256 free columns).  Softmax is unnormalized in scoresT
[sk, sq] orientation: exp(scale*s) on ACT, causal masking as a 0/1 multiply
on the exp output in SBUF (DVE/GpSimd), denominator via ones-column matmul
accumulated in PSUM, 1/den via a single custom-DVE reciprocal_approx_fast,
normalization via K=1 broadcast matmul + DVE multiply.

v2 scheduling notes:
  * startup DMAs are ordered critical-first per queue (first weight chunks
    and the first x tile ahead of everything; tables stream during sb0);
  * RoPE rotate-half is done with quadrant-shifted DVE reads against a
    pre-shifted sin table (no PE shift matmul, no PSUM rot tiles);
  * phase 2 runs one (sq block, head) section at a time: per kb the PE does
    scores + av + den (~640ns) while ACT does one exp (~720ns); deferred
    o-proj chains drain into the slack so the PE never waits on ACT;
  * 1/den runs on DVE (reciprocal_approx_fast), keeping ACT exp-only;
  * PSUM: scores ring 3 banks, av 2, den 1, bc/o-proj ring 2 = 8;
  * y is written bf16 on two DMA queues to shrink the output tail.
"""

import os

import numpy as np

S = 2048
D = 2048
HD = 128
NQH = 16
NKVH = 4
GROUPS = NQH // NKVH  # 4 q heads per kv head
O = GROUPS * HD  # 512 per-core q/o slice
NB = 2
NCORES = 8
SCALE = 1.0 / float(np.sqrt(np.float32(HD)))

SBLK = 512  # seq block for projections / sq block in attention
NKB = S // HD  # 16 128-blocks along seq
NSB = S // SBLK  # 4 512-blocks along seq
NDB = D // HD  # 16 d blocks

LAST_EXEC_NS = None
LAST_TRACE = None

_CACHE = {}


def _rope_tables():
    k = np.arange(0, HD, 2)[: HD // 2].astype(np.float32)
    inv_freq = (1.0 / 10000.0 ** (k / HD)).astype(np.float32)
    positions = np.arange(S, dtype=np.float32)
    ang = positions[:, None] * inv_freq[None, :]  # [S, 64]
    ang = np.concatenate([ang, ang], axis=-1)  # [S, 128]
    cosT = np.cos(ang).astype(np.float32).T  # [128, S]
    sinT = np.sin(ang).astype(np.float32).T
    return np.ascontiguousarray(cosT), np.ascontiguousarray(sinT)


def _shift_table():
    # rot = P @ q  with rot[i] = -q[i+64] (i<64), q[i-64] (i>=64); ship P.T
    P = np.zeros((HD, HD), dtype=np.float32)
    h = HD // 2
    P[np.arange(h), np.arange(h) + h] = -1.0
    P[np.arange(h) + h, np.arange(h)] = 1.0
    return np.ascontiguousarray(P.T)


def _mask_table():
    # binary keep-mask: maskT[i, j*512 + s] = 1 if (j*128 + i) <= s else 0
    m = np.empty((HD, 4 * SBLK), dtype=np.float32)
    i = np.arange(HD)[:, None]
    s = np.arange(SBLK)[None, :]
    for j in range(4):
        m[:, j * SBLK : (j + 1) * SBLK] = np.where(j * HD + i <= s, 1.0, 0.0)
    return m


def _build_program(split_waits=True):
    import concourse.bass as bass
    import concourse.mybir as mybir
    from concourse.tile import TileContext

    f32 = mybir.dt.float32
    f32r = mybir.dt.float32r
    bf16 = mybir.dt.bfloat16
    EXP = mybir.ActivationFunctionType.Exp
    LN = mybir.ActivationFunctionType.Ln

    nc = bass.Bass()

    xP = nc.declare_dram_parameter("xP", [NSB * 4 * 128, 4 * SBLK], bf16, isOutput=False)
    wqP = nc.declare_dram_parameter("wqP", [NDB * 128, O], bf16, isOutput=False)
    wkP = nc.declare_dram_parameter("wkP", [128, NDB * HD], bf16, isOutput=False)
    wvP = nc.declare_dram_parameter("wvP", [128, NDB * HD], bf16, isOutput=False)
    woP = nc.declare_dram_parameter("woP", [128, GROUPS * D], bf16, isOutput=False)
    cosB = nc.declare_dram_parameter("cosB", [HD, S], bf16, isOutput=False)
    sinB = nc.declare_dram_parameter("sinB", [HD, S], bf16, isOutput=False)
    shiftPT = nc.declare_dram_parameter("shiftPT", [HD, HD], f32r, isOutput=False)
    maskT = nc.declare_dram_parameter("maskT", [HD, 4 * SBLK], f32, isOutput=False)
    ident = nc.declare_dram_parameter("ident", [HD, HD], f32r, isOutput=False)
    onescol = nc.declare_dram_parameter("onescol", [HD, 1], f32r, isOutput=False)
    onesrow = nc.declare_dram_parameter("onesrow", [1, HD], f32r, isOutput=False)
    y = nc.declare_dram_parameter("y", [S, D], bf16, isOutput=True)

    with TileContext(nc) as tc:
        with tc.tile_pool(name="persist", bufs=1) as pp:
            wq_sb = pp.tile([128, NDB * O], bf16, name="wq_sb")  # [d_blk][128d, 512o]
            wk_sb = pp.tile([128, NDB * HD], bf16, name="wk_sb")
            wv_sb = pp.tile([128, NDB * HD], bf16, name="wv_sb")
            wo_sb = pp.tile([128, GROUPS * D], bf16, name="wo_sb")  # [o_blk][128o, 2048]
            cos_sb = pp.tile([128, S], f32, name="cos_sb")
            sin_sb = pp.tile([128, S], f32, name="sin_sb")
            shift_sb = pp.tile([128, HD], f32r, name="shift_sb")
            mask_sb = pp.tile([128, 4 * SBLK], f32, name="mask_sb")
            id_sb = pp.tile([128, HD], f32r, name="id_sb")
            ones_sb = pp.tile([128, 1], f32r, name="ones_sb")
            oner_sb = pp.tile([1, HD], f32r, name="oner_sb")
            # per-sb tiles so phase-2 readers only depend on the blocks
            # they actually touch (Tile tracks deps at tile granularity)
            q_sbs = [
                pp.tile([128, GROUPS * SBLK], f32r, name=f"q_sb{sb}")
                for sb in range(NSB)
            ]  # [h][128hd, 512s]
            k_sbs = [pp.tile([128, SBLK], f32r, name=f"k_sb{sb}") for sb in range(NSB)]
            v_sbs = [
                pp.tile([128, 4 * HD], f32r, name=f"v_sb{sb}") for sb in range(NSB)
            ]  # [sub][128s, 128hd]

            # Critical prefix on the scalar queue: exactly what the first
            # few projection blocks need; everything else streams during sb0.
            nc.scalar.dma_start(out=wq_sb[:, 0:O], in_=wqP[0:128, :])
            nc.scalar.dma_start(out=wk_sb[:, 0 : 4 * HD], in_=wkP[:, 0 : 4 * HD])
            nc.scalar.dma_start(out=wv_sb[:, 0 : 4 * HD], in_=wvP[:, 0 : 4 * HD])

            # ---------------- Phase 1: projections + RoPE + v transpose
            with (
                tc.tile_pool(name="p1acc", bufs=6, space="PSUM") as accp,
                tc.tile_pool(name="p1rot", bufs=2, space="PSUM") as rotp,
                tc.tile_pool(name="xts", bufs=6) as xpool,
                tc.tile_pool(name="raws", bufs=4) as rawpool,
                tc.tile_pool(name="tmps", bufs=2) as tmppool,
            ):
                p1_pending = []
                p2_warm = []
                rope_pool = [rotp]
                tmp_pool = [tmppool]

                def p1_flush(n=99):
                    nonlocal p1_pending
                    k = min(n, len(p1_pending))
                    for f in p1_pending[:k]:
                        f()
                    p1_pending = p1_pending[k:]

                for sb in range(NSB):
                    sl = slice(sb * SBLK, (sb + 1) * SBLK)
                    ps = [
                        accp.tile([128, SBLK], f32, name=f"acc{i}_{sb}", tag="acc")
                        for i in range(6)
                    ]  # q0..q3, k, v
                    xt4 = None
                    for db in range(NDB):
                        # staged weight/table DMAs, queue-ordered critical-first
                        if sb == 0:
                            if db == 1:
                                nc.scalar.dma_start(
                                    out=wq_sb[:, O : 4 * O],
                                    in_=wqP[128 : 4 * 128, :].rearrange(
                                        "(g p) c -> p g c", g=3
                                    ),
                                )
                            elif db == 2:
                                nc.scalar.dma_start(
                                    out=wq_sb[:, 4 * O : 8 * O],
                                    in_=wqP[4 * 128 : 8 * 128, :].rearrange(
                                        "(g p) c -> p g c", g=4
                                    ),
                                )
                            elif db == 3:
                                nc.scalar.dma_start(
                                    out=wk_sb[:, 4 * HD :],
                                    in_=wkP[:, 4 * HD :],
                                )
                                nc.scalar.dma_start(
                                    out=wv_sb[:, 4 * HD :],
                                    in_=wvP[:, 4 * HD :],
                                )
                            elif db == 6:
                                nc.scalar.dma_start(
                                    out=wq_sb[:, 8 * O : 12 * O],
                                    in_=wqP[8 * 128 : 12 * 128, :].rearrange(
                                        "(g p) c -> p g c", g=4
                                    ),
                                )
                            elif db == 8:
                                nc.scalar.dma_start(
                                    out=wq_sb[:, 12 * O : 16 * O],
                                    in_=wqP[12 * 128 : 16 * 128, :].rearrange(
                                        "(g p) c -> p g c", g=4
                                    ),
                                )
                            elif db == 10:
                                cosb = xpool.tile([128, S], bf16, name="cosb", tag="xt")
                                nc.scalar.dma_start(out=cosb[:], in_=cosB[:])
                                nc.vector.tensor_copy(cos_sb[:], cosb[:])
                            elif db == 11:
                                sinb = xpool.tile([128, S], bf16, name="sinb", tag="xt")
                                nc.scalar.dma_start(out=sinb[:], in_=sinB[:])
                                nc.vector.tensor_copy(sin_sb[:], sinb[:])
                            elif db == 13:
                                nc.scalar.dma_start(out=shift_sb[:], in_=shiftPT[:])
                                nc.scalar.dma_start(out=id_sb[:], in_=ident[:])
                                nc.scalar.dma_start(out=ones_sb[:], in_=onescol[:])
                                nc.scalar.dma_start(out=oner_sb[:], in_=onesrow[:])
                        elif sb == 1:
                            if db in (2, 6, 10, 14):
                                c = (db - 2) // 4
                                nc.scalar.dma_start(
                                    out=wo_sb[:, c * 2048 : (c + 1) * 2048],
                                    in_=woP[:, c * 2048 : (c + 1) * 2048],
                                )
                            elif db == 12:
                                nc.scalar.dma_start(out=mask_sb[:], in_=maskT[:])
                        if db % 4 == 0:
                            g = db // 4
                            xt4 = xpool.tile(
                                [128, 4 * SBLK], bf16, name=f"xt{sb}_{g}", tag="xt"
                            )
                            xq = nc.sync if (sb == 0 or g % 2 == 0) else nc.scalar
                            blk = (sb * 4 + g) * 128
                            if sb == 0 and g == 0:
                                xq.dma_start(
                                    out=xt4[:, 0:SBLK], in_=xP[blk : blk + 128, 0:SBLK]
                                )
                                xq.dma_start(
                                    out=xt4[:, SBLK:], in_=xP[blk : blk + 128, SBLK:]
                                )
                            else:
                                xq.dma_start(out=xt4[:], in_=xP[blk : blk + 128, :])
                        xt = xt4[:, (db % 4) * SBLK : (db % 4 + 1) * SBLK]
                        st = db == 0
                        sp = db == NDB - 1
                        if db > 1 and db % 2 == 0:
                            p1_flush(1)
                        for ob in range(GROUPS):
                            nc.tensor.matmul(
                                ps[ob][:],
                                wq_sb[:, db * O + ob * 128 : db * O + (ob + 1) * 128],
                                xt,
                                start=st,
                                stop=sp,
                            )
                        nc.tensor.matmul(
                            ps[4][:],
                            wk_sb[:, db * HD : (db + 1) * HD],
                            xt,
                            start=st,
                            stop=sp,
                        )
                        nc.tensor.matmul(
                            ps[5][:],
                            wv_sb[:, db * HD : (db + 1) * HD],
                            xt,
                            start=st,
                            stop=sp,
                        )

                    # PSUM -> SBUF copies spread across ACT/DVE (GpSimd
                    # cannot read PSUM) so the acc banks free up fast.
                    if sb == NSB - 1:
                        vst = pp.tile([128, SBLK], f32r, name=f"vst{sb}")
                    else:
                        vst = rawpool.tile([128, SBLK], f32r, name=f"vst{sb}", tag="vst", bufs=2)
                    nc.vector.tensor_copy(vst[:], ps[5][:])
                    raws = []
                    for i in range(5):
                        if sb == NSB - 1:
                            raw = pp.tile([128, SBLK], f32r, name=f"raw{sb}_{i}")
                        else:
                            raw = rawpool.tile([128, SBLK], f32r, name=f"raw{sb}_{i}", tag="raw", bufs=5)
                        if i % 2 == 0:
                            nc.scalar.copy(raw[:], ps[i][:])
                        else:
                            nc.vector.tensor_copy(raw[:], ps[i][:])
                        raws.append(raw)

                    # RoPE rotate (PE shift matmul) + v transposes go on the
                    # phase-1 work queue, drained one per db of the next sb so
                    # the PE never idles at block boundaries.  t2 = rot*sin on
                    # DVE (reads PSUM), t1 = raw*cos and adds on GpSimd.
                    def make_rope(sb, i, raw, sl=sl):
                        def run():
                            rot = rope_pool[0].tile(
                                [128, SBLK], f32, name=f"rot{sb}_{i}", tag="ring"
                            )
                            nc.tensor.matmul(
                                rot[:], shift_sb[:], raw[:], start=True, stop=True
                            )
                            dst = (
                                q_sbs[sb][:, i * SBLK : (i + 1) * SBLK]
                                if i < 4
                                else k_sbs[sb][:]
                            )
                            t1 = tmp_pool[0].tile(
                                [128, SBLK], f32, name=f"t1_{sb}_{i}", tag="t1", bufs=3
                            )
                            nc.gpsimd.tensor_mul(t1[:], raw[:], cos_sb[:, sl])
                            t2 = tmp_pool[0].tile(
                                [128, SBLK], f32, name=f"t2_{sb}_{i}", tag="t2", bufs=3
                            )
                            nc.vector.tensor_mul(t2[:], rot[:], sin_sb[:, sl])
                            (nc.gpsimd if i % 2 == 0 else nc.vector).tensor_add(
                                dst, t1[:], t2[:]
                            )
                        return run

                    def make_vt(sb, sub, vst=vst):
                        def run():
                            vt = rope_pool[0].tile(
                                [128, SBLK], f32r, name=f"vt{sb}_{sub}", tag="ring"
                            )
                            nc.tensor.transpose(
                                vt[:, :HD], vst[:, sub * HD : (sub + 1) * HD], id_sb[:]
                            )
                            nc.scalar.copy(
                                v_sbs[sb][:, sub * HD : (sub + 1) * HD], vt[:, :HD]
                            )
                        return run

                    if sb == NSB - 1:
                        # the last block's rope drains INSIDE phase 2 (its
                        # outputs are only read by the late sq=3 sections) so
                        # the PE never idles across the phase boundary; the
                        # vt transposes are PE work that pads the tail.
                        for sub in range(SBLK // HD):
                            p1_pending.append(make_vt(sb, sub))
                        for i in range(5):
                            p2_warm.append(make_rope(sb, i, raws[i]))
                    else:
                        for i in range(5):
                            p1_pending.append(make_rope(sb, i, raws[i]))
                        for sub in range(SBLK // HD):
                            p1_pending.append(make_vt(sb, sub))
                p1_flush()

            # ---------------- Phase 2: attention + o-proj
            # kb-PAIRED softmax: scores for two kb blocks land in one
            # 2-bank PSUM tile and a single ACT exp covers both (amortizes
            # the ~290ns ACT per-op overhead).  PSUM: pair ring 2x2 banks,
            # av 1, den 1, bc/o-proj/rope ring 2 = 8.
            with (
                tc.tile_pool(name="p2pair", bufs=2, space="PSUM") as pairp,
                tc.tile_pool(name="p2av", bufs=1, space="PSUM") as avp,
                tc.tile_pool(name="p2den", bufs=1, space="PSUM") as denp,
                tc.tile_pool(name="p2ring", bufs=2, space="PSUM") as ringp,
                tc.tile_pool(name="exps", bufs=3) as epool,
                tc.tile_pool(name="denrs", bufs=2) as drpool,
                tc.tile_pool(name="ysb", bufs=3) as ypool_sb,
                tc.tile_pool(name="ystash", bufs=16) as ystash,
            ):
                rope_pool[0] = ringp
                tmp_pool[0] = drpool
                ao_bufs = [
                    pp.tile([128, GROUPS * SBLK], bf16, name=f"aobuf{i}")
                    for i in range(2)
                ]
                # deferred work drained into the following sections'
                # instruction streams: epilogues (hi) promptly, o-proj
                # chains (lo) as PE filler while ACT streams exps.
                pending_hi = []
                pending_lo = []  # (required epi_drained, closure)
                epi = [0, 0]  # emitted, drained

                def flush_hi(n=99):
                    nonlocal pending_hi
                    k = min(n, len(pending_hi))
                    for f in pending_hi[:k]:
                        f()
                    pending_hi = pending_hi[k:]
                    epi[1] += k

                def flush_lo(n):
                    nonlocal pending_lo
                    k = 0
                    while k < n and k < len(pending_lo) and pending_lo[k][0] <= epi[1]:
                        pending_lo[k][1]()
                        k += 1
                    pending_lo = pending_lo[k:]

                def drain(n):
                    # warm items (last sb's rope) first, then ready chains
                    k = min(n, len(p2_warm))
                    for f in p2_warm[:k]:
                        f()
                    del p2_warm[:k]
                    flush_lo(n - k)

                def make_chain(sq, sub, dc, ao, obs=(0, 1, 2, 3), stash=None):
                    def run():
                        yt = ringp.tile(
                            [128, SBLK], f32, name=f"y{sq}_{sub}_{dc}", tag="ring"
                        )
                        for ob in obs:
                            nc.tensor.matmul(
                                yt[:],
                                ao[:, ob * SBLK + sub * HD : ob * SBLK + (sub + 1) * HD],
                                wo_sb[:, ob * D + dc * SBLK : ob * D + (dc + 1) * SBLK],
                                start=(ob == obs[0]),
                                stop=(ob == obs[-1]),
                            )
                        if stash is not None and stash[0] is None:
                            # first half of a split chain: park it in SBUF
                            ys = ystash.tile(
                                [128, SBLK], bf16, name=f"yh{sub}_{dc}", tag="yh"
                            )
                            nc.vector.tensor_copy(ys[:], yt[:])
                            stash[0] = ys
                            return
                        ysb = ypool_sb.tile(
                            [128, SBLK], bf16, name=f"ysb{sq}_{sub}_{dc}", tag="ysb"
                        )
                        if stash is not None:
                            nc.vector.tensor_add(ysb[:], yt[:], stash[0][:])
                        else:
                            nc.vector.tensor_copy(ysb[:], yt[:])
                        yq = nc.sync if dc % 2 == 0 else nc.scalar
                        yq.dma_start(
                            out=y[
                                sq * SBLK + sub * HD : sq * SBLK + (sub + 1) * HD,
                                dc * SBLK : (dc + 1) * SBLK,
                            ],
                            in_=ysb[:],
                        )
                    return run

                # sq0 (2 pair iterations per head) is too thin to hide its
                # own seams; run it last so sq3's o-proj chains pad it, and
                # split its o-proj by head pair so only half the chains land
                # after the final section.
                for sq in (1, 2, 3, 0):
                    nsk = 4 * sq + 4
                    npr = nsk // 2
                    lag = 2 if npr >= 4 else 1
                    ao = ao_bufs[sq % 2]
                    stashes = {}
                    for h in range(GROUPS):
                        dent = denp.tile(
                            [1, SBLK], f32, name=f"den{sq}_{h}", tag="den"
                        )
                        avt = avp.tile(
                            [128, SBLK], f32, name=f"av{sq}_{h}", tag="av"
                        )
                        es = {}

                        def emit_avden(p, avt=avt, dent=dent, es=es, nsk=nsk):
                            e = es.pop(p)
                            for half in range(2):
                                kb = 2 * p + half
                                st = kb == 0
                                sp = kb == nsk - 1
                                esl = e[:, half * SBLK : (half + 1) * SBLK]
                                nc.tensor.matmul(
                                    avt[:],
                                    v_sbs[kb // 4][:, (kb % 4) * HD : (kb % 4 + 1) * HD],
                                    esl,
                                    start=st,
                                    stop=sp,
                                )
                                nc.tensor.matmul(
                                    dent[:],
                                    ones_sb[:],
                                    esl,
                                    start=st,
                                    stop=sp,
                                )

                        for p in range(npr):
                            pair = pairp.tile(
                                [128, 2 * SBLK], f32, name=f"pr{sq}_{h}_{p}", tag="pair"
                            )
                            for half in range(2):
                                kb = 2 * p + half
                                nc.tensor.matmul(
                                    pair[:, half * SBLK : (half + 1) * SBLK],
                                    k_sbs[kb // 4][:, (kb % 4) * HD : (kb % 4 + 1) * HD],
                                    q_sbs[sq][:, h * SBLK : (h + 1) * SBLK],
                                    start=True,
                                    stop=True,
                                )
                            e = epool.tile(
                                [128, 2 * SBLK], f32r, name=f"e{sq}_{h}_{p}", tag="e"
                            )
                            nc.scalar.activation(e[:], pair[:], EXP, scale=SCALE)
                            if 2 * p >= 4 * sq:
                                # causal mask on both halves at once; the
                                # final pair is latency-critical - keep it
                                # off the slow GpSimd
                                j = 2 * p - 4 * sq
                                meng = (
                                    nc.vector
                                    if (j or h % 2 == 0)
                                    else nc.gpsimd
                                )
                                meng.tensor_mul(
                                    e[:], e[:], mask_sb[:, j * SBLK : (j + 2) * SBLK]
                                )
                            es[p] = e
                            if p == 1:
                                flush_hi(1)
                            elif sq == 0:
                                drain(2)
                            elif len(pending_lo) > 8 or p % 2 == 0:
                                drain(1)
                            if p > lag - 1:
                                emit_avden(p - lag)
                        for p in range(npr - lag, npr):
                            emit_avden(p)
                            if p < npr - 1:
                                drain(1)
                        # stage av out of PSUM on ACT so the bank frees for
                        # the next section immediately
                        avsb = drpool.tile(
                            [128, SBLK], f32, name=f"avsb{h}", tag="avsb", bufs=2
                        )
                        nc.scalar.copy(avsb[:], avt[:])

                        def epilogue(h=h, dent=dent, avsb=avsb, ao=ao):
                            # 1/den on ACT: denr = exp(-ln(den)); ln+exp live
                            # in the same ACT table set, so no table reloads.
                            denr = drpool.tile(
                                [1, SBLK], f32r, name=f"denr{h}", tag="denr", bufs=2
                            )
                            nc.scalar.activation(denr[:], dent[:], LN)
                            nc.scalar.activation(denr[:], denr[:], EXP, scale=-1.0)
                            bct = pairp.tile(
                                [128, 2 * SBLK], f32, name=f"bc{h}", tag="pair"
                            )
                            bc = bct[:, 0:SBLK]
                            nc.tensor.matmul(
                                bc,
                                oner_sb[:],
                                denr[:],
                                start=True,
                                stop=True,
                            )
                            # TensorTensor may read only one PSUM operand:
                            # stage the broadcast through SBUF.
                            bcs = drpool.tile(
                                [128, SBLK], f32, name=f"bcs{h}", tag="bcs", bufs=2
                            )
                            nc.vector.tensor_copy(bcs[:], bc)
                            nc.vector.tensor_mul(
                                ao[:, h * SBLK : (h + 1) * SBLK], avsb[:], bcs[:]
                            )

                        pending_hi.append(epilogue)
                        epi[0] += 1

                        if sq == 0 and h == 1:
                            # first half of the final block's o-proj: heads
                            # 0-1 are final once their epilogues drain
                            flush_hi()
                            for sub in range(SBLK // HD):
                                for dc in range(D // SBLK):
                                    stash = [None]
                                    stashes[(sub, dc)] = stash
                                    pending_lo.append(
                                        (epi[0], make_chain(sq, sub, dc, ao, (0, 1), stash))
                                    )

                    # o-proj chains for this sq block drain inside the next
                    # block's attention stream.
                    if sq == 0:
                        flush_hi()
                        for sub in range(SBLK // HD):
                            for dc in range(D // SBLK):
                                pending_lo.append(
                                    (epi[0], make_chain(
                                        sq, sub, dc, ao, (2, 3), stashes[(sub, dc)]
                                    ))
                                )
                    else:
                        for sub in range(SBLK // HD):
                            for dc in range(D // SBLK):
                                pending_lo.append(
                                    (epi[0], make_chain(sq, sub, dc, ao))
                                )
                flush_hi()
                drain(999)
    if split_waits:
        _split_matmul_waits(nc, mybir)
    return nc


def _split_matmul_waits(nc, mybir):
    """TRN2 instructions can carry only one HW sync-wait command; Tile
    sometimes attaches several.  Move the extras onto nofuse nops on the
    same engine inserted just before the instruction."""
    for f in nc.m.functions:
        for bb in f.blocks:
            insts = bb.instructions
            fixes = []
            for idx, inst in enumerate(insts):
                si = inst.sync_info
                if si is None or len(si.on_wait) <= 1:
                    continue
                fixes.append((idx, inst, list(si.on_wait), list(si.on_update)))
            for idx, inst, waits, updates in reversed(fixes):
                inst.sync_info = mybir.SyncInfo(on_wait=[waits[-1]], on_update=updates)
                for w in reversed(waits[:-1]):
                    nop = mybir.InstNoOp(
                        name=nc.get_next_instruction_name(), ins=[], outs=[]
                    )
                    nop.engine = inst.engine
                    nop.bass_nofuse = True
                    nop.sync_info = mybir.SyncInfo(on_wait=[w], on_update=[])
                    insts.insert(idx, nop)


def _per_core_inputs(x, Wq, Wk, Wv, Wo):
    import ml_dtypes

    bf16 = ml_dtypes.bfloat16
    cosT, sinT = _rope_tables()
    shiftPT = _shift_table()
    maskT = _mask_table()
    ident = np.eye(HD, dtype=np.float32)
    onescol = np.ones((HD, 1), dtype=np.float32)
    onesrow = np.ones((1, HD), dtype=np.float32)
    in_maps = []
    for b in range(NB):
        xTb = x[b].T.astype(bf16)  # [D, S]
        # pack into contiguous [sb][db][128, 512] blocks for sequential DMA
        xPb = np.ascontiguousarray(
            xTb.reshape(4, 4, 128, NSB, SBLK)      # [g, db_in_g, 128, sb, 512]
            .transpose(3, 0, 2, 1, 4)              # [sb, g, 128, db_in_g, 512]
            .reshape(NSB * 4 * 128, 4 * SBLK)
        )
        for g in range(NKVH):
            wqT = Wq[g * O : (g + 1) * O, :].T  # [D, O]
            wkT = Wk[g * HD : (g + 1) * HD, :].T
            wvT = Wv[g * HD : (g + 1) * HD, :].T
            woT = Wo[:, g * O : (g + 1) * O].T  # [O, D]
            in_maps.append(
                {
                    "xP": xPb,
                    "wqP": np.ascontiguousarray(
                        wqT.reshape(NDB * 128, O).astype(bf16)
                    ),
                    "wkP": np.ascontiguousarray(
                        wkT.reshape(NDB, 128, HD).transpose(1, 0, 2).reshape(128, NDB * HD).astype(bf16)
                    ),
                    "wvP": np.ascontiguousarray(
                        wvT.reshape(NDB, 128, HD).transpose(1, 0, 2).reshape(128, NDB * HD).astype(bf16)
                    ),
                    "woP": np.ascontiguousarray(
                        woT.reshape(GROUPS, 128, D).transpose(1, 0, 2).reshape(128, GROUPS * D).astype(bf16)
                    ),
                    "cosB": cosT.astype(bf16),
                    "sinB": sinT.astype(bf16),
                    "shiftPT": shiftPT,
                    "maskT": maskT,
                    "ident": ident,
                    "onescol": onescol,
                    "onesrow": onesrow,
                }
            )
    return in_maps


def kernel(x, Wq, Wk, Wv, Wo):
    global LAST_EXEC_NS, LAST_TRACE
    from concourse.bass_utils import run_bass_kernel_spmd

    if "nc" not in _CACHE:
        _CACHE["nc"] = _build_program()
    nc = _CACHE["nc"]

    x = np.asarray(x)
    in_maps = _per_core_inputs(
        x, np.asarray(Wq), np.asarray(Wk), np.asarray(Wv), np.asarray(Wo)
    )
    trace = bool(os.environ.get("KERNEL_PROFILE"))
    res = run_bass_kernel_spmd(
        nc, in_maps, core_ids=list(range(NCORES)), trace=trace
    )
    globals()["LAST_RESULT"] = res
    LAST_EXEC_NS = res.exec_time_ns
    LAST_TRACE = getattr(res, "profile_json", None)
    out = np.empty((NB, S, D), dtype=np.float32)
    for b in range(NB):
        acc = res.results[b * NKVH]["y"].astype(np.float32)
        for g in range(1, NKVH):
            acc = acc + res.results[b * NKVH + g]["y"].astype(np.float32)
        out[b] = acc
    return out
